# revision 9
# baseline (speedup 1.0000x reference)
"""DeltaRule (diagonal-state linear attention) Bass kernel for 8 TRN2 cores.

Problem: nn_DeltaRule_20194936225992
  B=4, S=2048, H_DIM=1024, N_HEADS=16, HEAD_DIM=64.
  q/k/v/b projections, phi = elu+1, per-(b,h,d) scalar linear recurrence
      s_t = (1 - b_t*pk_t^2) * s_{t-1} + b_t*v_t*pk_t ;  y_t = s_t * pq_t
  out = y @ Wo.T + bo

Sharding: core = (batch b, head-group hg) with hg covering 8 heads.
Each core computes its partial O-projection (contraction over its 512
lanes); host sums the two head-group partials per batch, transposes
[o,t] -> [t,o] and adds bo.

v2 design (weight-stationary):
  Time is processed in 2 halves of SH=1024.  Each projection (proj, lt)
  is ONE compound matmul group per half: psum tile [128, 2, 512] (2
  banks), 8 d-steps, each step = 1 LDWEIGHTS + 2 N=512 MATMULs.  This
  amortizes the 128-row weight load over 1024 output columns (11% PE
  overhead instead of 25%).

  Engine split per (half, lt), ops on [128, 1024]:
    PE:  K, V, Q projection groups; O-projection groups per ot.
    ACT: rk=relu(psk+bk), r2k=relu(-psk-bk), ek=exp(-r2k)  (k-side phi)
         rq, r2q, eq (q-side phi, bf16), O-proj PSUM->SBUF copies (bf16)
    DVE: pk=ek+rk, w=pk*b, cc=(psv+bv)*w (fused STT from PSUM),
         a=1-g (tensor_scalar 2x mode), scan, pq=eq+rq (bf16 2x),
         y=s*pq (bf16 out)
    GPS: g = pk*w  (gpsimd; SBUF-only ops)
  The v-bias is folded into cc's scalar_tensor_tensor (no ones-row
  matmul); the sigmoid gate b is computed on the host (0.4% of FLOPs)
  and DMA'd pre-broadcast per lane.  Output partials are written bf16.
"""

import os
import sys

for _p in ("/opt/trn_rl_repo", os.path.expanduser("~/.axon_site/_ro/trn_rl_repo")):
    if os.path.isdir(_p) and _p not in sys.path:
        sys.path.insert(0, _p)

import numpy as np  # noqa: E402

import concourse.bass as bass  # noqa: E402
import concourse.tile as tile  # noqa: E402
from concourse import bacc, mybir  # noqa: E402
from concourse.bass import ts  # noqa: E402
from concourse.bass_utils import run_bass_kernel_spmd  # noqa: E402

# problem constants (hardcoded per task rules)
B, S, H_DIM, N_HEADS, HEAD_DIM = 4, 2048, 1024, 16, 64
P = 128
NCORES = 8
HG = 2                      # head groups
J = 512                     # lanes per core  (8 heads * 64)
JT = J // P                 # 4 j-tiles
DT = H_DIM // P             # 8 contraction tiles
HPC = N_HEADS // HG         # 8 heads per core
SH = 1024                   # half length
NH = S // SH                # 2 halves
TC = 512                    # psum chunk (1 bank)
CPH = SH // TC              # chunks per half = 2

IN_DT_NAME = os.environ.get("DELTA_IN_DT", "bfloat16")

F32 = mybir.dt.float32
BF16 = mybir.dt.bfloat16
AF = mybir.ActivationFunctionType


def build_nc(in_dt_name=None):
    if in_dt_name is None:
        in_dt_name = IN_DT_NAME
    in_dt = getattr(mybir.dt, in_dt_name)

    nc = bacc.Bacc(trn_type="TRN2", target_bir_lowering=False, debug=False)

    # per-core inputs (host-packed; see make_in_maps)
    xq = nc.dram_tensor("xq", [P, DT, S], in_dt, kind="ExternalInput").ap()
    xk = nc.dram_tensor("xk", [P, DT, S], in_dt, kind="ExternalInput").ap()
    xv = nc.dram_tensor("xv", [P, DT, S], in_dt, kind="ExternalInput").ap()
    bbb = nc.dram_tensor("bbb", [P, JT, S], in_dt, kind="ExternalInput").ap()
    wq = nc.dram_tensor("wq", [P, JT, DT, P], in_dt, kind="ExternalInput").ap()
    wk = nc.dram_tensor("wk", [P, JT, DT, P], in_dt, kind="ExternalInput").ap()
    wv = nc.dram_tensor("wv", [P, JT, DT, P], in_dt, kind="ExternalInput").ap()
    wo = nc.dram_tensor("wo", [P, JT, H_DIM], in_dt, kind="ExternalInput").ap()
    bq = nc.dram_tensor("bq", [P, JT], F32, kind="ExternalInput").ap()
    bk = nc.dram_tensor("bk", [P, JT], F32, kind="ExternalInput").ap()
    nbq = nc.dram_tensor("nbq", [P, JT], F32, kind="ExternalInput").ap()
    nbk = nc.dram_tensor("nbk", [P, JT], F32, kind="ExternalInput").ap()
    bvl = nc.dram_tensor("bvl", [P, JT], F32, kind="ExternalInput").ap()
    out = nc.dram_tensor("out", [H_DIM, S], in_dt, kind="ExternalOutput").ap()

    from contextlib import ExitStack

    M = mybir.AluOpType

    with tile.TileContext(nc) as tcx, ExitStack() as ctx:
        wpool = ctx.enter_context(tcx.tile_pool(name="weights", bufs=1))
        xpool = ctx.enter_context(tcx.tile_pool(name="xin", bufs=1))
        ipool = ctx.enter_context(tcx.tile_pool(name="inter", bufs=2))
        ipool1 = ctx.enter_context(tcx.tile_pool(name="inter1", bufs=1))
        spool = ctx.enter_context(tcx.tile_pool(name="scan", bufs=1))
        cpool = ctx.enter_context(tcx.tile_pool(name="scol", bufs=2))
        ypool = ctx.enter_context(tcx.tile_pool(name="ysb", bufs=2))
        opool = ctx.enter_context(tcx.tile_pool(name="osb", bufs=2))
        ppool = ctx.enter_context(tcx.tile_pool(name="ps", bufs=4, space="PSUM"))

        # --- persistent weights / constants ---
        wk_sb = wpool.tile([P, JT, DT, P], in_dt, tag="wk")
        wv_sb = wpool.tile([P, JT, DT, P], in_dt, tag="wv")
        wq_sb = wpool.tile([P, JT, DT, P], in_dt, tag="wq")
        wo_sb = wpool.tile([P, JT, H_DIM], in_dt, tag="wo")
        bq_sb = wpool.tile([P, JT], F32, tag="bq")
        bk_sb = wpool.tile([P, JT], F32, tag="bk")
        nbq_sb = wpool.tile([P, JT], F32, tag="nbq")
        nbk_sb = wpool.tile([P, JT], F32, tag="nbk")
        bvl_sb = wpool.tile([P, JT], F32, tag="bvl")

        # head: first k-group needs only wk[lt=0] (0.25MB) + xk half0 (2MB)
        nc.sync.dma_start(out=wk_sb[:, 0], in_=wk[:, 0])

        s_prev = [None] * JT    # per-lt scan tile of previous half

        for h in range(NH):
            hsl = ts(h, SH)

            # --- stream x for this half (per-tensor; WAR dep on last read) ---
            xk_c = xpool.tile([P, DT, CPH, TC], in_dt, tag="xk")
            nc.sync.dma_start(out=xk_c[:], in_=xk[:, :, hsl])
            if h == 0:
                nc.sync.dma_start(out=bk_sb[:], in_=bk)
                nc.sync.dma_start(out=nbk_sb[:], in_=nbk)
                for lt in range(1, JT):
                    nc.sync.dma_start(out=wk_sb[:, lt], in_=wk[:, lt])
                nc.sync.dma_start(out=wv_sb[:], in_=wv)
            xv_c = xpool.tile([P, DT, CPH, TC], in_dt, tag="xv")
            nc.sync.dma_start(out=xv_c[:], in_=xv[:, :, hsl])
            bb_c = xpool.tile([P, JT, SH], in_dt, tag="bbb")
            nc.sync.dma_start(out=bb_c[:], in_=bbb[:, :, hsl])
            if h == 0:
                nc.sync.dma_start(out=bvl_sb[:], in_=bvl)
                nc.sync.dma_start(out=wq_sb[:], in_=wq)
                nc.sync.dma_start(out=bq_sb[:], in_=bq)
                nc.sync.dma_start(out=nbq_sb[:], in_=nbq)
            xq_c = xpool.tile([P, DT, CPH, TC], in_dt, tag="xq")
            nc.sync.dma_start(out=xq_c[:], in_=xq[:, :, hsl])
            if h == 0:
                nc.sync.dma_start(out=wo_sb[:], in_=wo)

            y_t = []
            for lt in range(JT):
                # ---- k projection group ----
                psk = ppool.tile([P, CPH, TC], F32, tag="ps")
                for d in range(DT):
                    for c in range(CPH):
                        nc.tensor.matmul(
                            out=psk[:, c], lhsT=wk_sb[:, lt, d],
                            rhs=xk_c[:, d, c],
                            start=(d == 0), stop=(d == DT - 1),
                        )
                # ---- v projection group ----
                psv = ppool.tile([P, CPH, TC], F32, tag="ps")
                for d in range(DT):
                    for c in range(CPH):
                        nc.tensor.matmul(
                            out=psv[:, c], lhsT=wv_sb[:, lt, d],
                            rhs=xv_c[:, d, c],
                            start=(d == 0), stop=(d == DT - 1),
                        )

                # ---- phi(k): rk + exp(-r2k) ----
                rk = ipool.tile([P, SH], F32, tag="rpos")
                nc.scalar.activation(out=rk[:], in_=psk[:], func=AF.Relu,
                                     bias=bk_sb[:, lt:lt + 1])
                r2k = ipool1.tile([P, SH], F32, tag="rneg")
                nc.scalar.activation(out=r2k[:], in_=psk[:], func=AF.Relu,
                                     bias=nbk_sb[:, lt:lt + 1], scale=-1.0)
                ek = ipool.tile([P, SH], F32, tag="ex")
                nc.scalar.activation(out=ek[:], in_=r2k[:], func=AF.Exp,
                                     scale=-1.0)
                pk = ipool.tile([P, SH], F32, tag="pk")
                nc.vector.tensor_tensor(out=pk[:], in0=ek[:], in1=rk[:], op=M.add)

                # ---- gates: w = pk*b ; cc = (v+bv)*w ; g = pk*w ; a = 1-g ----
                w = ipool.tile([P, SH], F32, tag="w")
                nc.vector.tensor_tensor(out=w[:], in0=pk[:], in1=bb_c[:, lt],
                                        op=M.mult)
                cc = ipool.tile([P, SH], F32, tag="cc")
                nc.vector.scalar_tensor_tensor(
                    out=cc[:], in0=psv[:], scalar=bvl_sb[:, lt:lt + 1],
                    in1=w[:], op0=M.add, op1=M.mult)
                g = ipool.tile([P, SH], F32, tag="g")
                nc.gpsimd.tensor_tensor(out=g[:], in0=pk[:], in1=w[:], op=M.mult)
                a = ipool.tile([P, SH], F32, tag="a")
                nc.vector.tensor_scalar(out=a[:], in0=g[:], scalar1=-1.0,
                                        scalar2=1.0, op0=M.mult, op1=M.add)

                # ---- the recurrence: s = a*s_prev + cc along time ----
                s_new = spool.tile([P, SH], F32, tag=f"s{lt}")
                init = 0.0 if h == 0 else s_prev[lt][:]
                nc.vector.tensor_tensor_scan(
                    out=s_new[:], data0=a[:], data1=cc[:], initial=init,
                    op0=M.mult, op1=M.add,
                )
                if h < NH - 1:
                    scol = cpool.tile([P, 1], F32, tag=f"sc{lt}")
                    nc.vector.tensor_scalar(out=scol[:],
                                            in0=s_new[:, SH - 1:SH],
                                            scalar1=0.0, scalar2=None,
                                            op0=M.add)
                    s_prev[lt] = scol

                # ---- q projection group ----
                psq = ppool.tile([P, CPH, TC], F32, tag="ps")
                for d in range(DT):
                    for c in range(CPH):
                        nc.tensor.matmul(
                            out=psq[:, c], lhsT=wq_sb[:, lt, d],
                            rhs=xq_c[:, d, c],
                            start=(d == 0), stop=(d == DT - 1),
                        )
                # ---- phi(q) in bf16 + y = s * pq ----
                rq = ipool.tile([P, SH], BF16, tag="rposq")
                nc.scalar.activation(out=rq[:], in_=psq[:], func=AF.Relu,
                                     bias=bq_sb[:, lt:lt + 1])
                r2q = ipool1.tile([P, SH], BF16, tag="rnegq")
                nc.scalar.activation(out=r2q[:], in_=psq[:], func=AF.Relu,
                                     bias=nbq_sb[:, lt:lt + 1], scale=-1.0)
                eq = ipool.tile([P, SH], BF16, tag="exq")
                nc.scalar.activation(out=eq[:], in_=r2q[:], func=AF.Exp,
                                     scale=-1.0)
                pq = ipool.tile([P, SH], BF16, tag="pq")
                nc.vector.tensor_tensor(out=pq[:], in0=eq[:], in1=rq[:], op=M.add)
                y = ypool.tile([P, CPH, TC], in_dt, tag=f"y{lt}")
                nc.vector.tensor_tensor(out=y[:], in0=s_new[:], in1=pq[:],
                                        op=M.mult)
                y_t.append(y)

            # ---- O projection: out[o, t] += wo[j, o] * y[j, t] ----
            for ot in range(DT):
                pso = ppool.tile([P, CPH, TC], F32, tag="ps")
                for lt in range(JT):
                    for c in range(CPH):
                        nc.tensor.matmul(
                            out=pso[:, c], lhsT=wo_sb[:, lt, ts(ot, P)],
                            rhs=y_t[lt][:, c],
                            start=(lt == 0), stop=(lt == JT - 1),
                        )
                o_sb = opool.tile([P, SH], in_dt, tag="osb")
                nc.scalar.copy(out=o_sb[:], in_=pso[:])
                nc.sync.dma_start(out=out[ts(ot, P), hsl], in_=o_sb[:])

    nc.compile()
    return nc


_NC_CACHE = {}


def _get_nc():
    key = IN_DT_NAME
    if key not in _NC_CACHE:
        _NC_CACHE[key] = build_nc()
    return _NC_CACHE[key]


def _np_in_dt():
    if IN_DT_NAME == "bfloat16":
        import ml_dtypes
        return ml_dtypes.bfloat16
    return np.float32


def make_in_maps(query, key, value, beta, Wq, bq, Wk, bk, Wv, bv, Wb, bb, Wo, bo):
    """Host-side shard prep: core_id = b*2 + hg."""
    ndt = _np_in_dt()

    def xpack(x):  # [S, H_DIM] -> [p, dt, t] in in_dt
        a = np.asarray(x, np.float32).T            # [H_DIM, S] = [dt*128+p, t]
        a = a.reshape(DT, P, S).transpose(1, 0, 2)  # [p, dt, t]
        return np.ascontiguousarray(a).astype(ndt)

    def wpack(Wsl):  # [J, H_DIM] -> [p, lt, dt, m]
        a = np.asarray(Wsl, np.float32)            # [lt*128+m, dt*128+p]
        a = a.reshape(JT, P, DT, P)                # [lt, m, dt, p]
        a = a.transpose(3, 0, 2, 1)                # [p, lt, dt, m]
        return np.ascontiguousarray(a).astype(ndt)

    xqs = [xpack(query[b]) for b in range(B)]
    xks = [xpack(key[b]) for b in range(B)]
    xvs = [xpack(value[b]) for b in range(B)]
    # gate b computed host-side (0.4% of FLOPs), pre-broadcast per lane
    Wbf = np.asarray(Wb, np.float32)
    bbf0 = np.asarray(bb, np.float32)
    z = np.einsum('bsd,hd->bsh', np.asarray(beta, np.float32), Wbf) + bbf0
    bgate = 1.0 / (1.0 + np.exp(-z))                      # [B, S, 16]

    def bpack(bl):  # [S, J] -> [p, lt, t]
        a = bl.T.reshape(JT, P, S)                        # [lt, p, t]
        return np.ascontiguousarray(a.transpose(1, 0, 2)).astype(ndt)

    bqf = np.asarray(bq, np.float32)
    bkf = np.asarray(bk, np.float32)
    bvf = np.asarray(bv, np.float32)

    in_maps = []
    for b in range(B):
        for hg in range(HG):
            jsl = slice(hg * J, (hg + 1) * J)
            hsl = slice(hg * HPC, (hg + 1) * HPC)

            def lanes(v):  # [J] -> [128, 4] per lane-tile columns
                return np.ascontiguousarray(v[jsl].reshape(JT, P).T)

            wop = np.asarray(Wo, np.float32)[:, jsl]      # [o, j]
            wop = wop.T.reshape(JT, P, H_DIM).transpose(1, 0, 2)  # [p, lt, o]

            in_maps.append({
                "xq": xqs[b], "xk": xks[b], "xv": xvs[b],
                "bbb": bpack(np.repeat(bgate[b][:, hsl], HEAD_DIM, axis=1)),
                "wq": wpack(Wq[jsl]), "wk": wpack(Wk[jsl]),
                "wv": wpack(Wv[jsl]),
                "wo": np.ascontiguousarray(wop).astype(ndt),
                "bq": lanes(bqf), "bk": lanes(bkf),
                "nbq": lanes(-bqf), "nbk": lanes(-bkf),
                "bvl": lanes(bvf),
            })
    return in_maps


LAST_RESULTS = None


def kernel(**inputs):
    global LAST_RESULTS
    nc = _get_nc()
    in_maps = make_in_maps(**inputs)
    res = run_bass_kernel_spmd(nc, in_maps, core_ids=list(range(NCORES)),
                               trace=bool(os.environ.get("DELTA_TRACE")))
    LAST_RESULTS = res
    bo = np.asarray(inputs["bo"], np.float32)
    out = np.empty((B, S, H_DIM), np.float32)
    for b in range(B):
        m = (np.asarray(res.results[2 * b]["out"], np.float32)
             + np.asarray(res.results[2 * b + 1]["out"], np.float32))
        out[b] = m.T + bo
    return out


# revision 16
# speedup vs baseline: 1.0513x; 1.0513x over previous
"""DeltaRule (diagonal-state linear attention) Bass kernel for 8 TRN2 cores.

Problem: nn_DeltaRule_20194936225992
  B=4, S=2048, H_DIM=1024, N_HEADS=16, HEAD_DIM=64.
  q/k/v/b projections, phi = elu+1, per-(b,h,d) scalar linear recurrence
      s_t = (1 - b_t*pk_t^2) * s_{t-1} + b_t*v_t*pk_t ;  y_t = s_t * pq_t
  out = y @ Wo.T + bo

Sharding: core = (batch b, head-group hg) with hg covering 8 heads.
Each core computes its partial O-projection (contraction over its 512
lanes); host sums the two head-group partials per batch (bf16 partials),
transposes [o,t] -> [t,o] and adds bo.

v3 design: time processed in 2 halves of SH=1024; ops on [128,1024]
tiles.  PE schedule per half: k-proj for all 4 lane-tiles first (covers
the DMA ramp: k needs only wk+xk), then v/q pairs per lane-tile, then
the O-projection in ot-pairs with the lt=3 contribution deferred so the
last y tile is off the critical path.  PSUM groups are [128,2,512]
(2 banks, 16 interleaved per-bank matmuls).

  Engine split per (half, lt):
    ACT: rk=relu(psk+bk), r2k=relu(-psk-bk), ek=exp(-r2k)  (k-side phi)
         rq, r2q, eq (q-side phi, bf16), half the O-copies (bf16)
    DVE: pk=ek+rk, w=pk*b, cc=(psv+bv)*w (fused STT from PSUM),
         a=1-g (tensor_scalar 2x), scan, pq=eq+rq (bf16 2x), y=s*pq,
         half the O-copies
    GPS: g = pk*w
  The v-bias is folded into cc's scalar_tensor_tensor; the sigmoid gate
  b is computed on the host (0.4% of FLOPs), DMA'd pre-broadcast.
"""

import os
import sys

for _p in ("/opt/trn_rl_repo", os.path.expanduser("~/.axon_site/_ro/trn_rl_repo")):
    if os.path.isdir(_p) and _p not in sys.path:
        sys.path.insert(0, _p)

import numpy as np  # noqa: E402

import concourse.bass as bass  # noqa: E402
import concourse.tile as tile  # noqa: E402
from concourse import bacc, mybir  # noqa: E402
from concourse.bass import ts  # noqa: E402
from concourse.bass_utils import run_bass_kernel_spmd  # noqa: E402

# problem constants (hardcoded per task rules)
B, S, H_DIM, N_HEADS, HEAD_DIM = 4, 2048, 1024, 16, 64
P = 128
NCORES = 8
HG = 2                      # head groups
J = 512                     # lanes per core  (8 heads * 64)
JT = J // P                 # 4 j-tiles
DT = H_DIM // P             # 8 contraction tiles
HPC = N_HEADS // HG         # 8 heads per core
SH = 1024                   # half length
NH = S // SH                # 2 halves
TC = 512                    # psum chunk (1 bank)
CPH = SH // TC              # chunks per half = 2

IN_DT_NAME = os.environ.get("DELTA_IN_DT", "bfloat16")

F32 = mybir.dt.float32
BF16 = mybir.dt.bfloat16
AF = mybir.ActivationFunctionType


def build_nc(in_dt_name=None):
    if in_dt_name is None:
        in_dt_name = IN_DT_NAME
    in_dt = getattr(mybir.dt, in_dt_name)

    nc = bacc.Bacc(trn_type="TRN2", target_bir_lowering=False, debug=False)

    # per-core inputs (host-packed; see make_in_maps)
    xq = nc.dram_tensor("xq", [P, DT, S], in_dt, kind="ExternalInput").ap()
    xk = nc.dram_tensor("xk", [P, DT, S], in_dt, kind="ExternalInput").ap()
    xv = nc.dram_tensor("xv", [P, DT, S], in_dt, kind="ExternalInput").ap()
    bbb = nc.dram_tensor("bbb", [P, JT, S], in_dt, kind="ExternalInput").ap()
    wq = nc.dram_tensor("wq", [P, JT, DT, P], in_dt, kind="ExternalInput").ap()
    wk = nc.dram_tensor("wk", [P, JT, DT, P], in_dt, kind="ExternalInput").ap()
    wv = nc.dram_tensor("wv", [P, JT, DT, P], in_dt, kind="ExternalInput").ap()
    wo = nc.dram_tensor("wo", [P, JT, H_DIM], in_dt, kind="ExternalInput").ap()
    bq = nc.dram_tensor("bq", [P, JT], F32, kind="ExternalInput").ap()
    bk = nc.dram_tensor("bk", [P, JT], F32, kind="ExternalInput").ap()
    nbq = nc.dram_tensor("nbq", [P, JT], F32, kind="ExternalInput").ap()
    nbk = nc.dram_tensor("nbk", [P, JT], F32, kind="ExternalInput").ap()
    bvl = nc.dram_tensor("bvl", [P, JT], F32, kind="ExternalInput").ap()
    out = nc.dram_tensor("out", [H_DIM, S], in_dt, kind="ExternalOutput").ap()

    from contextlib import ExitStack

    M = mybir.AluOpType

    with tile.TileContext(nc) as tcx, ExitStack() as ctx:
        wpool = ctx.enter_context(tcx.tile_pool(name="weights", bufs=1))
        xpool = ctx.enter_context(tcx.tile_pool(name="xin", bufs=1))
        ipool = ctx.enter_context(tcx.tile_pool(name="inter", bufs=2))
        ipool1 = ctx.enter_context(tcx.tile_pool(name="inter1", bufs=1))
        spool = ctx.enter_context(tcx.tile_pool(name="scan", bufs=1))
        cpool = ctx.enter_context(tcx.tile_pool(name="scol", bufs=2))
        ypool = ctx.enter_context(tcx.tile_pool(name="ysb", bufs=2))
        opool = ctx.enter_context(tcx.tile_pool(name="osb", bufs=2))
        ppool = ctx.enter_context(tcx.tile_pool(name="ps", bufs=4, space="PSUM"))

        # --- persistent weights / constants ---
        wk_sb = wpool.tile([P, JT, DT, P], in_dt, tag="wk")
        wv_sb = wpool.tile([P, JT, DT, P], in_dt, tag="wv")
        wq_sb = wpool.tile([P, JT, DT, P], in_dt, tag="wq")
        wo_sb = wpool.tile([P, JT, H_DIM], in_dt, tag="wo")
        bq_sb = wpool.tile([P, JT], F32, tag="bq")
        bk_sb = wpool.tile([P, JT], F32, tag="bk")
        nbq_sb = wpool.tile([P, JT], F32, tag="nbq")
        nbk_sb = wpool.tile([P, JT], F32, tag="nbk")
        bvl_sb = wpool.tile([P, JT], F32, tag="bvl")

        # head: tiny bias loads, first k weights, first xk chunk
        nc.sync.dma_start(out=bk_sb[:], in_=bk)
        nc.sync.dma_start(out=nbk_sb[:], in_=nbk)
        for lt in range(JT):
            nc.sync.dma_start(out=wk_sb[:, lt], in_=wk[:, lt])

        s_prev = [None] * JT    # per-lt last-column state of previous half

        def load_x(src, tag, h):
            """Per-512-chunk tiles for fine-grained DMA->matmul overlap."""
            tl = []
            for c in range(CPH):
                t_ = xpool.tile([P, DT, TC], in_dt, tag=f"{tag}{c}")
                nc.sync.dma_start(out=t_[:], in_=src[:, :, ts(h * CPH + c, TC)])
                tl.append(t_)
            return tl

        for h in range(NH):
            hsl = ts(h, SH)

            if h == 0:
                # --- stream x for the first half ---
                xk_c = load_x(xk, "xk", 0)
                nc.sync.dma_start(out=wv_sb[:], in_=wv)
                xv_c = load_x(xv, "xv", 0)
                bb_c = xpool.tile([P, JT, SH], in_dt, tag="bbb")
                nc.sync.dma_start(out=bb_c[:], in_=bbb[:, :, hsl])
                nc.sync.dma_start(out=bvl_sb[:], in_=bvl)
                nc.sync.dma_start(out=wq_sb[:], in_=wq)
                nc.sync.dma_start(out=bq_sb[:], in_=bq)
                nc.sync.dma_start(out=nbq_sb[:], in_=nbq)
                xq_c = load_x(xq, "xq", 0)
                nc.sync.dma_start(out=wo_sb[:], in_=wo)

            # ---- k projections for all lane-tiles (DMA ramp cover) ----
            psk_t = []
            for lt in range(JT):
                psk = ppool.tile([P, CPH, TC], F32, tag="ps")
                for d in range(DT):
                    for c in range(CPH):
                        nc.tensor.matmul(
                            out=psk[:, c], lhsT=wk_sb[:, lt, d],
                            rhs=xk_c[c][:, d],
                            start=(d == 0), stop=(d == DT - 1),
                        )
                psk_t.append(psk)
                # phi(k) drains free the psum tile quickly
                rk = ipool.tile([P, SH], F32, tag="rpos")
                nc.scalar.activation(out=rk[:], in_=psk[:], func=AF.Relu,
                                     bias=bk_sb[:, lt:lt + 1])
                r2k = ipool1.tile([P, SH], F32, tag="rneg")
                nc.scalar.activation(out=r2k[:], in_=psk[:], func=AF.Relu,
                                     bias=nbk_sb[:, lt:lt + 1], scale=-1.0)
                ek = ipool.tile([P, SH], F32, tag="ex")
                nc.scalar.activation(out=ek[:], in_=r2k[:], func=AF.Exp,
                                     scale=-1.0)
                pk = ipool1.tile([P, SH], F32, tag=f"pk{lt}")
                nc.vector.tensor_tensor(out=pk[:], in0=ek[:], in1=rk[:],
                                        op=M.add)
                psk_t[lt] = pk

            # all xk reads emitted; prefetch next half's behind them
            if h < NH - 1:
                xk_n = load_x(xk, "xk", h + 1)

            y_t = []
            for lt in range(JT):
                pk = psk_t[lt]
                # ---- v projection ----
                psv = ppool.tile([P, CPH, TC], F32, tag="ps")
                for d in range(DT):
                    for c in range(CPH):
                        nc.tensor.matmul(
                            out=psv[:, c], lhsT=wv_sb[:, lt, d],
                            rhs=xv_c[c][:, d],
                            start=(d == 0), stop=(d == DT - 1),
                        )
                # ---- gates + recurrence ----
                w = ipool.tile([P, SH], F32, tag="w")
                nc.vector.tensor_tensor(out=w[:], in0=pk[:], in1=bb_c[:, lt],
                                        op=M.mult)
                g = ipool.tile([P, SH], F32, tag="g")
                nc.gpsimd.tensor_tensor(out=g[:], in0=pk[:], in1=w[:],
                                        op=M.mult)
                cc = ipool.tile([P, SH], F32, tag="cc")
                nc.vector.scalar_tensor_tensor(
                    out=cc[:], in0=psv[:], scalar=bvl_sb[:, lt:lt + 1],
                    in1=w[:], op0=M.add, op1=M.mult)
                a = ipool.tile([P, SH], F32, tag="a")
                nc.vector.tensor_scalar(out=a[:], in0=g[:], scalar1=-1.0,
                                        scalar2=1.0, op0=M.mult, op1=M.add)
                s_new = spool.tile([P, SH], F32, tag=f"s{lt}")
                init = 0.0 if h == 0 else s_prev[lt][:]
                nc.vector.tensor_tensor_scan(
                    out=s_new[:], data0=a[:], data1=cc[:], initial=init,
                    op0=M.mult, op1=M.add,
                )
                if h < NH - 1:
                    scol = cpool.tile([P, 1], F32, tag=f"sc{lt}")
                    nc.vector.tensor_scalar(out=scol[:],
                                            in0=s_new[:, SH - 1:SH],
                                            scalar1=0.0, scalar2=None,
                                            op0=M.add)
                    s_prev[lt] = scol

                # ---- q projection ----
                psq = ppool.tile([P, CPH, TC], F32, tag="ps")
                for d in range(DT):
                    for c in range(CPH):
                        nc.tensor.matmul(
                            out=psq[:, c], lhsT=wq_sb[:, lt, d],
                            rhs=xq_c[c][:, d],
                            start=(d == 0), stop=(d == DT - 1),
                        )
                # ---- phi(q) in bf16 + y = s * pq ----
                rq = ipool.tile([P, SH], BF16, tag="rposq")
                nc.scalar.activation(out=rq[:], in_=psq[:], func=AF.Relu,
                                     bias=bq_sb[:, lt:lt + 1])
                r2q = ipool1.tile([P, SH], BF16, tag="rnegq")
                nc.scalar.activation(out=r2q[:], in_=psq[:], func=AF.Relu,
                                     bias=nbq_sb[:, lt:lt + 1], scale=-1.0)
                eq = ipool.tile([P, SH], BF16, tag="exq")
                nc.scalar.activation(out=eq[:], in_=r2q[:], func=AF.Exp,
                                     scale=-1.0)
                pq = ipool.tile([P, SH], BF16, tag="pq")
                nc.vector.tensor_tensor(out=pq[:], in0=eq[:], in1=rq[:],
                                        op=M.add)
                y = ypool.tile([P, CPH, TC], in_dt, tag=f"y{lt}")
                nc.vector.tensor_tensor(out=y[:], in0=s_new[:], in1=pq[:],
                                        op=M.mult)
                y_t.append(y)

            # all xv / xq / bbb reads emitted; prefetch next half's
            if h < NH - 1:
                xv_n = load_x(xv, "xv", h + 1)
                bb_n = xpool.tile([P, JT, SH], in_dt, tag="bbb")
                nc.sync.dma_start(out=bb_n[:], in_=bbb[:, :, ts(h + 1, SH)])
                xq_n = load_x(xq, "xq", h + 1)

            # ---- O projection: out[o,t] += wo[j,o]*y[j,t], lt3 deferred ----
            for op_ in range(DT // 2):
                pts = []
                for ot in (2 * op_, 2 * op_ + 1):
                    pso = ppool.tile([P, CPH, TC], F32, tag="ps")
                    for lt in range(JT - 1):
                        for c in range(CPH):
                            nc.tensor.matmul(
                                out=pso[:, c], lhsT=wo_sb[:, lt, ts(ot, P)],
                                rhs=y_t[lt][:, c],
                                start=(lt == 0), stop=False,
                            )
                    pts.append(pso)
                for i, ot in enumerate((2 * op_, 2 * op_ + 1)):
                    pso = pts[i]
                    for c in range(CPH):
                        nc.tensor.matmul(
                            out=pso[:, c], lhsT=wo_sb[:, JT - 1, ts(ot, P)],
                            rhs=y_t[JT - 1][:, c],
                            start=False, stop=True,
                        )
                    o_sb = opool.tile([P, SH], in_dt, tag="osb")
                    if ot % 2 == 0:
                        nc.scalar.copy(out=o_sb[:], in_=pso[:])
                    else:
                        nc.vector.tensor_scalar(out=o_sb[:], in0=pso[:],
                                                scalar1=0.0, scalar2=None,
                                                op0=M.add)
                    nc.sync.dma_start(out=out[ts(ot, P), hsl], in_=o_sb[:])

            if h < NH - 1:
                xk_c, xv_c, bb_c, xq_c = xk_n, xv_n, bb_n, xq_n

    nc.compile()
    return nc


_NC_CACHE = {}


def _get_nc():
    key = IN_DT_NAME
    if key not in _NC_CACHE:
        _NC_CACHE[key] = build_nc()
    return _NC_CACHE[key]


def _np_in_dt():
    if IN_DT_NAME == "bfloat16":
        import ml_dtypes
        return ml_dtypes.bfloat16
    return np.float32


def make_in_maps(query, key, value, beta, Wq, bq, Wk, bk, Wv, bv, Wb, bb, Wo, bo):
    """Host-side shard prep: core_id = b*2 + hg."""
    ndt = _np_in_dt()

    def xpack(x):  # [S, H_DIM] -> [p, dt, t] in in_dt
        a = np.asarray(x, np.float32).T            # [H_DIM, S] = [dt*128+p, t]
        a = a.reshape(DT, P, S).transpose(1, 0, 2)  # [p, dt, t]
        return np.ascontiguousarray(a).astype(ndt)

    def wpack(Wsl):  # [J, H_DIM] -> [p, lt, dt, m]
        a = np.asarray(Wsl, np.float32)            # [lt*128+m, dt*128+p]
        a = a.reshape(JT, P, DT, P)                # [lt, m, dt, p]
        a = a.transpose(3, 0, 2, 1)                # [p, lt, dt, m]
        return np.ascontiguousarray(a).astype(ndt)

    xqs = [xpack(query[b]) for b in range(B)]
    xks = [xpack(key[b]) for b in range(B)]
    xvs = [xpack(value[b]) for b in range(B)]
    # gate b computed host-side (0.4% of FLOPs), pre-broadcast per lane
    Wbf = np.asarray(Wb, np.float32)
    bbf0 = np.asarray(bb, np.float32)
    z = np.einsum('bsd,hd->bsh', np.asarray(beta, np.float32), Wbf) + bbf0
    bgate = 1.0 / (1.0 + np.exp(-z))                      # [B, S, 16]

    def bpack(bl):  # [S, J] -> [p, lt, t]
        a = bl.T.reshape(JT, P, S)                        # [lt, p, t]
        return np.ascontiguousarray(a.transpose(1, 0, 2)).astype(ndt)

    bqf = np.asarray(bq, np.float32)
    bkf = np.asarray(bk, np.float32)
    bvf = np.asarray(bv, np.float32)

    in_maps = []
    for b in range(B):
        for hg in range(HG):
            jsl = slice(hg * J, (hg + 1) * J)
            hsl = slice(hg * HPC, (hg + 1) * HPC)

            def lanes(v):  # [J] -> [128, 4] per lane-tile columns
                return np.ascontiguousarray(v[jsl].reshape(JT, P).T)

            wop = np.asarray(Wo, np.float32)[:, jsl]      # [o, j]
            wop = wop.T.reshape(JT, P, H_DIM).transpose(1, 0, 2)  # [p, lt, o]

            in_maps.append({
                "xq": xqs[b], "xk": xks[b], "xv": xvs[b],
                "bbb": bpack(np.repeat(bgate[b][:, hsl], HEAD_DIM, axis=1)),
                "wq": wpack(Wq[jsl]), "wk": wpack(Wk[jsl]),
                "wv": wpack(Wv[jsl]),
                "wo": np.ascontiguousarray(wop).astype(ndt),
                "bq": lanes(bqf), "bk": lanes(bkf),
                "nbq": lanes(-bqf), "nbk": lanes(-bkf),
                "bvl": lanes(bvf),
            })
    return in_maps


LAST_RESULTS = None


def kernel(**inputs):
    global LAST_RESULTS
    nc = _get_nc()
    in_maps = make_in_maps(**inputs)
    res = run_bass_kernel_spmd(nc, in_maps, core_ids=list(range(NCORES)),
                               trace=bool(os.environ.get("DELTA_TRACE")))
    LAST_RESULTS = res
    bo = np.asarray(inputs["bo"], np.float32)
    out = np.empty((B, S, H_DIM), np.float32)
    for b in range(B):
        m = (np.asarray(res.results[2 * b]["out"], np.float32)
             + np.asarray(res.results[2 * b + 1]["out"], np.float32))
        out[b] = m.T + bo
    return out


# revision 20
# speedup vs baseline: 1.1470x; 1.0910x over previous
"""DeltaRule (diagonal-state linear attention) Bass kernel for 8 TRN2 cores.

Problem: nn_DeltaRule_20194936225992
  B=4, S=2048, H_DIM=1024, N_HEADS=16, HEAD_DIM=64.
  q/k/v/b projections, phi = elu+1, per-(b,h,d) scalar linear recurrence
      s_t = (1 - b_t*pk_t^2) * s_{t-1} + b_t*v_t*pk_t ;  y_t = s_t * pq_t
  out = y @ Wo.T + bo

Sharding: core = (batch b, head-group hg) with hg covering 8 heads.
Each core computes its partial O-projection over its 512 lanes; host
sums the two bf16 head-group partials per batch, transposes and adds bo.

v4 design: time processed in 2 halves of SH=1024, elementwise ops on
[128,1024] tiles.  PE schedule per half:
  1. k-projections for all 4 lane-tiles (PSUM groups [128,2,512],
     c-outer so the first group only needs the first 512-col x chunk).
     The FULL gate chain (phi(k), w=pk*b, g=pk*w on gpsimd, a=1-g)
     drains per lane-tile right here, so by the v/q phase only cc and
     the scan remain on the critical path.
  2. v/q pairs per lane-tile: cc=(psv+bv)*w fused from PSUM, scan,
     phi(q) in bf16, y=s*pq.
  3. O-projection in ot-pairs with the lt=3 contribution deferred
     (last y stays off the critical path).  Copies alternate ACT/DVE.
DMA: head order puts xk chunk 0 first, then wk (single dispatch), then
the rest; biases ride in one merged [P,5,JT] tensor; next half's x
prefetches are emitted right after the last reader of this half's.
PE steady state measured ~216 ns per 512-col matmul (ldweights fully
overlapped) == bf16 roofline.
"""

import os
import sys

for _p in ("/opt/trn_rl_repo", os.path.expanduser("~/.axon_site/_ro/trn_rl_repo")):
    if os.path.isdir(_p) and _p not in sys.path:
        sys.path.insert(0, _p)

import numpy as np  # noqa: E402

import concourse.bass as bass  # noqa: E402
import concourse.tile as tile  # noqa: E402
from concourse import bacc, mybir  # noqa: E402
from concourse.bass import ts  # noqa: E402
from concourse.bass_utils import run_bass_kernel_spmd  # noqa: E402

# problem constants (hardcoded per task rules)
B, S, H_DIM, N_HEADS, HEAD_DIM = 4, 2048, 1024, 16, 64
P = 128
NCORES = 8
HG = 2                      # head groups
J = 512                     # lanes per core  (8 heads * 64)
JT = J // P                 # 4 j-tiles
DT = H_DIM // P             # 8 contraction tiles
HPC = N_HEADS // HG         # 8 heads per core
SH = 1024                   # half length
NH = S // SH                # 2 halves
TC = 512                    # psum chunk (1 bank)
CPH = SH // TC              # chunks per half = 2

IN_DT_NAME = os.environ.get("DELTA_IN_DT", "bfloat16")

F32 = mybir.dt.float32
BF16 = mybir.dt.bfloat16
AF = mybir.ActivationFunctionType


def build_nc(in_dt_name=None):
    if in_dt_name is None:
        in_dt_name = IN_DT_NAME
    in_dt = getattr(mybir.dt, in_dt_name)

    nc = bacc.Bacc(trn_type="TRN2", target_bir_lowering=False, debug=False)

    # per-core inputs (host-packed; see make_in_maps)
    xq = nc.dram_tensor("xq", [P, DT, S], in_dt, kind="ExternalInput").ap()
    xk = nc.dram_tensor("xk", [P, DT, S], in_dt, kind="ExternalInput").ap()
    xv = nc.dram_tensor("xv", [P, DT, S], in_dt, kind="ExternalInput").ap()
    bbb = nc.dram_tensor("bbb", [P, JT, S], in_dt, kind="ExternalInput").ap()
    wq = nc.dram_tensor("wq", [P, JT, DT, P], in_dt, kind="ExternalInput").ap()
    wk = nc.dram_tensor("wk", [P, JT, DT, P], in_dt, kind="ExternalInput").ap()
    wv = nc.dram_tensor("wv", [P, JT, DT, P], in_dt, kind="ExternalInput").ap()
    wo = nc.dram_tensor("wo", [P, JT, H_DIM], in_dt, kind="ExternalInput").ap()
    # merged biases: rows = (bq, bk, nbq, nbk, bvl)
    b5 = nc.dram_tensor("b5", [P, 5, JT], F32, kind="ExternalInput").ap()
    out = nc.dram_tensor("out", [H_DIM, S], in_dt, kind="ExternalOutput").ap()

    from contextlib import ExitStack

    M = mybir.AluOpType

    with tile.TileContext(nc) as tcx, ExitStack() as ctx:
        wpool = ctx.enter_context(tcx.tile_pool(name="weights", bufs=1))
        xpool = ctx.enter_context(tcx.tile_pool(name="xin", bufs=1))
        ipool = ctx.enter_context(tcx.tile_pool(name="inter", bufs=2))
        ipool1 = ctx.enter_context(tcx.tile_pool(name="inter1", bufs=1))
        spool = ctx.enter_context(tcx.tile_pool(name="scan", bufs=1))
        cpool = ctx.enter_context(tcx.tile_pool(name="scol", bufs=2))
        ypool = ctx.enter_context(tcx.tile_pool(name="ysb", bufs=2))
        opool = ctx.enter_context(tcx.tile_pool(name="osb", bufs=4))
        ppool = ctx.enter_context(tcx.tile_pool(name="ps", bufs=4, space="PSUM"))

        # --- persistent weights / constants ---
        wk_sb = wpool.tile([P, JT, DT, P], in_dt, tag="wk")
        wv_sb = wpool.tile([P, JT, DT, P], in_dt, tag="wv")
        wq_sb = wpool.tile([P, JT, DT, P], in_dt, tag="wq")
        wo_sb = wpool.tile([P, JT, H_DIM], in_dt, tag="wo")
        b5_sb = wpool.tile([P, 5, JT], F32, tag="b5")

        def bap(i, lt):  # [P,1] bias slice
            return b5_sb[:, i, lt:lt + 1]

        s_prev = [None] * JT    # per-lt last-column state of previous half

        def load_x(src, tag, h):
            """Per-512-chunk tiles for fine-grained DMA->matmul overlap."""
            tl = []
            for c in range(CPH):
                t_ = xpool.tile([P, DT, TC], in_dt, tag=f"{tag}{c}")
                nc.sync.dma_start(out=t_[:], in_=src[:, :, ts(h * CPH + c, TC)])
                tl.append(t_)
            return tl

        for h in range(NH):
            hsl = ts(h, SH)

            if h == 0:
                # --- head: xk chunk0 first, then wk, then the rest ---
                xk_c0 = xpool.tile([P, DT, TC], in_dt, tag="xk0", name="xk_c0")
                xk_c = [xk_c0]
                nc.sync.dma_start(out=xk_c[0][:], in_=xk[:, :, 0:TC])
                nc.sync.dma_start(out=wk_sb[:], in_=wk)
                t_ = xpool.tile([P, DT, TC], in_dt, tag="xk1")
                nc.sync.dma_start(out=t_[:], in_=xk[:, :, TC:SH])
                xk_c.append(t_)
                nc.sync.dma_start(out=b5_sb[:], in_=b5)
                bb_c = xpool.tile([P, JT, SH], in_dt, tag="bbb")
                nc.sync.dma_start(out=bb_c[:], in_=bbb[:, :, hsl])
                nc.sync.dma_start(out=wv_sb[:], in_=wv)
                xv_c = load_x(xv, "xv", 0)
                nc.sync.dma_start(out=wq_sb[:], in_=wq)
                xq_c = load_x(xq, "xq", 0)
                nc.sync.dma_start(out=wo_sb[:], in_=wo)

            # ---- phase 1: k projections + full gate chain per lane ----
            w_t = [None] * JT
            a_t = [None] * JT
            for lt in range(JT):
                psk = ppool.tile([P, CPH, TC], F32, tag="ps")
                for c in range(CPH):
                    for d in range(DT):
                        nc.tensor.matmul(
                            out=psk[:, c], lhsT=wk_sb[:, lt, d],
                            rhs=xk_c[c][:, d],
                            start=(d == 0), stop=(d == DT - 1),
                        )
                rk = ipool.tile([P, SH], F32, tag="rpos")
                nc.scalar.activation(out=rk[:], in_=psk[:], func=AF.Relu,
                                     bias=bap(1, lt))
                r2k = ipool1.tile([P, SH], F32, tag="rneg")
                nc.scalar.activation(out=r2k[:], in_=psk[:], func=AF.Relu,
                                     bias=bap(3, lt), scale=-1.0)
                ek = ipool1.tile([P, SH], F32, tag="ex")
                nc.scalar.activation(out=ek[:], in_=r2k[:], func=AF.Exp,
                                     scale=-1.0)
                pk = ipool.tile([P, SH], F32, tag="pk")
                nc.vector.tensor_tensor(out=pk[:], in0=ek[:], in1=rk[:],
                                        op=M.add)
                w = ipool1.tile([P, SH], F32, tag=f"w{lt}")
                nc.vector.tensor_tensor(out=w[:], in0=pk[:], in1=bb_c[:, lt],
                                        op=M.mult)
                g = ipool1.tile([P, SH], F32, tag="g")
                nc.gpsimd.tensor_tensor(out=g[:], in0=pk[:], in1=w[:],
                                        op=M.mult)
                a = ipool1.tile([P, SH], F32, tag=f"a{lt}")
                nc.vector.tensor_scalar(out=a[:], in0=g[:], scalar1=-1.0,
                                        scalar2=1.0, op0=M.mult, op1=M.add)
                w_t[lt] = w
                a_t[lt] = a

            # all xk reads emitted; prefetch next half's behind them
            if h < NH - 1:
                xk_n = load_x(xk, "xk", h + 1)

            # ---- phase 2: v/q pairs per lane-tile ----
            y_t = []
            for lt in range(JT):
                psv = ppool.tile([P, CPH, TC], F32, tag="ps")
                for d in range(DT):
                    for c in range(CPH):
                        nc.tensor.matmul(
                            out=psv[:, c], lhsT=wv_sb[:, lt, d],
                            rhs=xv_c[c][:, d],
                            start=(d == 0), stop=(d == DT - 1),
                        )
                cc = ipool.tile([P, SH], F32, tag="cc")
                nc.vector.scalar_tensor_tensor(
                    out=cc[:], in0=psv[:], scalar=bap(4, lt),
                    in1=w_t[lt][:], op0=M.add, op1=M.mult)
                s_new = spool.tile([P, SH], F32, tag=f"s{lt}")
                init = 0.0 if h == 0 else s_prev[lt][:]
                nc.vector.tensor_tensor_scan(
                    out=s_new[:], data0=a_t[lt][:], data1=cc[:], initial=init,
                    op0=M.mult, op1=M.add,
                )
                if h < NH - 1:
                    scol = cpool.tile([P, 1], F32, tag=f"sc{lt}")
                    nc.vector.tensor_scalar(out=scol[:],
                                            in0=s_new[:, SH - 1:SH],
                                            scalar1=0.0, scalar2=None,
                                            op0=M.add)
                    s_prev[lt] = scol

                psq = ppool.tile([P, CPH, TC], F32, tag="ps")
                for d in range(DT):
                    for c in range(CPH):
                        nc.tensor.matmul(
                            out=psq[:, c], lhsT=wq_sb[:, lt, d],
                            rhs=xq_c[c][:, d],
                            start=(d == 0), stop=(d == DT - 1),
                        )
                rq = ipool.tile([P, SH], BF16, tag="rposq")
                nc.scalar.activation(out=rq[:], in_=psq[:], func=AF.Relu,
                                     bias=bap(0, lt))
                r2q = ipool1.tile([P, SH], BF16, tag="rnegq")
                nc.scalar.activation(out=r2q[:], in_=psq[:], func=AF.Relu,
                                     bias=bap(2, lt), scale=-1.0)
                eq = ipool1.tile([P, SH], BF16, tag="exq")
                nc.scalar.activation(out=eq[:], in_=r2q[:], func=AF.Exp,
                                     scale=-1.0)
                pq = ipool1.tile([P, SH], BF16, tag="pq")
                nc.vector.tensor_tensor(out=pq[:], in0=eq[:], in1=rq[:],
                                        op=M.add)
                y = ypool.tile([P, CPH, TC], in_dt, tag=f"y{lt}")
                nc.vector.tensor_tensor(out=y[:], in0=s_new[:], in1=pq[:],
                                        op=M.mult)
                y_t.append(y)

            # all xv / xq / bbb reads emitted; prefetch next half's
            if h < NH - 1:
                xv_n = load_x(xv, "xv", h + 1)
                bb_n = xpool.tile([P, JT, SH], in_dt, tag="bbb")
                nc.sync.dma_start(out=bb_n[:], in_=bbb[:, :, ts(h + 1, SH)])
                xq_n = load_x(xq, "xq", h + 1)

            # ---- O projection in ot-pairs, lt=3 deferred ----
            for op_ in range(DT // 2):
                pts = []
                for ot in (2 * op_, 2 * op_ + 1):
                    pso = ppool.tile([P, CPH, TC], F32, tag="ps")
                    for lt in range(JT - 1):
                        for c in range(CPH):
                            nc.tensor.matmul(
                                out=pso[:, c], lhsT=wo_sb[:, lt, ts(ot, P)],
                                rhs=y_t[lt][:, c],
                                start=(lt == 0), stop=False,
                            )
                    pts.append(pso)
                for i, ot in enumerate((2 * op_, 2 * op_ + 1)):
                    pso = pts[i]
                    for c in range(CPH):
                        nc.tensor.matmul(
                            out=pso[:, c], lhsT=wo_sb[:, JT - 1, ts(ot, P)],
                            rhs=y_t[JT - 1][:, c],
                            start=False, stop=True,
                        )
                    o_sb = opool.tile([P, SH], in_dt, tag="osb")
                    if ot % 2 == 0:
                        nc.scalar.copy(out=o_sb[:], in_=pso[:])
                    else:
                        nc.vector.tensor_scalar(out=o_sb[:], in0=pso[:],
                                                scalar1=0.0, scalar2=None,
                                                op0=M.add)
                    nc.sync.dma_start(out=out[ts(ot, P), hsl], in_=o_sb[:])

            if h < NH - 1:
                xk_c, xv_c, bb_c, xq_c = xk_n, xv_n, bb_n, xq_n

    nc.compile()
    return nc


_NC_CACHE = {}


def _get_nc():
    key = IN_DT_NAME
    if key not in _NC_CACHE:
        _NC_CACHE[key] = build_nc()
    return _NC_CACHE[key]


def _np_in_dt():
    if IN_DT_NAME == "bfloat16":
        import ml_dtypes
        return ml_dtypes.bfloat16
    return np.float32


def make_in_maps(query, key, value, beta, Wq, bq, Wk, bk, Wv, bv, Wb, bb, Wo, bo):
    """Host-side shard prep: core_id = b*2 + hg."""
    ndt = _np_in_dt()

    def xpack(x):  # [S, H_DIM] -> [p, dt, t] in in_dt
        a = np.asarray(x, np.float32).T            # [H_DIM, S] = [dt*128+p, t]
        a = a.reshape(DT, P, S).transpose(1, 0, 2)  # [p, dt, t]
        return np.ascontiguousarray(a).astype(ndt)

    def wpack(Wsl):  # [J, H_DIM] -> [p, lt, dt, m]
        a = np.asarray(Wsl, np.float32)            # [lt*128+m, dt*128+p]
        a = a.reshape(JT, P, DT, P)                # [lt, m, dt, p]
        a = a.transpose(3, 0, 2, 1)                # [p, lt, dt, m]
        return np.ascontiguousarray(a).astype(ndt)

    xqs = [xpack(query[b]) for b in range(B)]
    xks = [xpack(key[b]) for b in range(B)]
    xvs = [xpack(value[b]) for b in range(B)]
    # gate b computed host-side (0.4% of FLOPs), pre-broadcast per lane
    Wbf = np.asarray(Wb, np.float32)
    bbf0 = np.asarray(bb, np.float32)
    z = np.einsum('bsd,hd->bsh', np.asarray(beta, np.float32), Wbf) + bbf0
    bgate = 1.0 / (1.0 + np.exp(-z))                      # [B, S, 16]

    def bpack(bl):  # [S, J] -> [p, lt, t]
        a = bl.T.reshape(JT, P, S)                        # [lt, p, t]
        return np.ascontiguousarray(a.transpose(1, 0, 2)).astype(ndt)

    bqf = np.asarray(bq, np.float32)
    bkf = np.asarray(bk, np.float32)
    bvf = np.asarray(bv, np.float32)

    in_maps = []
    for b in range(B):
        for hg in range(HG):
            jsl = slice(hg * J, (hg + 1) * J)
            hsl = slice(hg * HPC, (hg + 1) * HPC)

            def lanes(v):  # [J] -> [128, 4] per lane-tile columns
                return np.ascontiguousarray(v[jsl].reshape(JT, P).T)

            b5m = np.stack([lanes(bqf), lanes(bkf), lanes(-bqf),
                            lanes(-bkf), lanes(bvf)], axis=1)  # [P,5,JT]

            wop = np.asarray(Wo, np.float32)[:, jsl]      # [o, j]
            wop = wop.T.reshape(JT, P, H_DIM).transpose(1, 0, 2)  # [p, lt, o]

            in_maps.append({
                "xq": xqs[b], "xk": xks[b], "xv": xvs[b],
                "bbb": bpack(np.repeat(bgate[b][:, hsl], HEAD_DIM, axis=1)),
                "wq": wpack(Wq[jsl]), "wk": wpack(Wk[jsl]),
                "wv": wpack(Wv[jsl]),
                "wo": np.ascontiguousarray(wop).astype(ndt),
                "b5": np.ascontiguousarray(b5m),
            })
    return in_maps


LAST_RESULTS = None


def kernel(**inputs):
    global LAST_RESULTS
    nc = _get_nc()
    in_maps = make_in_maps(**inputs)
    res = run_bass_kernel_spmd(nc, in_maps, core_ids=list(range(NCORES)),
                               trace=bool(os.environ.get("DELTA_TRACE")))
    LAST_RESULTS = res
    bo = np.asarray(inputs["bo"], np.float32)
    out = np.empty((B, S, H_DIM), np.float32)
    for b in range(B):
        m = (np.asarray(res.results[2 * b]["out"], np.float32)
             + np.asarray(res.results[2 * b + 1]["out"], np.float32))
        out[b] = m.T + bo
    return out


# revision 24
# speedup vs baseline: 1.1591x; 1.0105x over previous
"""DeltaRule (diagonal-state linear attention) Bass kernel for 8 TRN2 cores.

Problem: nn_DeltaRule_20194936225992
  B=4, S=2048, H_DIM=1024, N_HEADS=16, HEAD_DIM=64.
  q/k/v/b projections, phi = elu+1, per-(b,h,d) scalar linear recurrence
      s_t = (1 - b_t*pk_t^2) * s_{t-1} + b_t*v_t*pk_t ;  y_t = s_t * pq_t
  out = y @ Wo.T + bo

Sharding: core = (batch b, head-group hg) with hg covering 8 heads.
Each core computes its partial O-projection over its 512 lanes; host
sums the two bf16 head-group partials per batch, transposes and adds bo.

v4 design: time processed in 2 halves of SH=1024, elementwise ops on
[128,1024] tiles.  PE schedule per half:
  1. k-projections for all 4 lane-tiles (PSUM groups [128,2,512],
     c-outer so the first group only needs the first 512-col x chunk).
     The FULL gate chain (phi(k), w=pk*b, g=pk*w on gpsimd, a=1-g)
     drains per lane-tile right here, so by the v/q phase only cc and
     the scan remain on the critical path.
  2. v/q pairs per lane-tile: cc=(psv+bv)*w fused from PSUM, scan,
     phi(q) in bf16, y=s*pq.
  3. O-projection in ot-pairs with the lt=3 contribution deferred
     (last y stays off the critical path).  Copies alternate ACT/DVE.
DMA: head order puts xk chunk 0 first, then wk (single dispatch), then
the rest; biases ride in one merged [P,5,JT] tensor; next half's x
prefetches are emitted right after the last reader of this half's.
PE steady state measured ~216 ns per 512-col matmul (ldweights fully
overlapped) == bf16 roofline.
"""

import os
import sys

for _p in ("/opt/trn_rl_repo", os.path.expanduser("~/.axon_site/_ro/trn_rl_repo")):
    if os.path.isdir(_p) and _p not in sys.path:
        sys.path.insert(0, _p)

import numpy as np  # noqa: E402

import concourse.bass as bass  # noqa: E402
import concourse.tile as tile  # noqa: E402
from concourse import bacc, mybir  # noqa: E402
from concourse.bass import ts  # noqa: E402
from concourse.bass_utils import run_bass_kernel_spmd  # noqa: E402

# problem constants (hardcoded per task rules)
B, S, H_DIM, N_HEADS, HEAD_DIM = 4, 2048, 1024, 16, 64
P = 128
NCORES = 8
HG = 2                      # head groups
J = 512                     # lanes per core  (8 heads * 64)
JT = J // P                 # 4 j-tiles
DT = H_DIM // P             # 8 contraction tiles
HPC = N_HEADS // HG         # 8 heads per core
SH = 1024                   # half length
NH = S // SH                # 2 halves
TC = 512                    # psum chunk (1 bank)
CPH = SH // TC              # chunks per half = 2

IN_DT_NAME = os.environ.get("DELTA_IN_DT", "bfloat16")

F32 = mybir.dt.float32
BF16 = mybir.dt.bfloat16
AF = mybir.ActivationFunctionType


def build_nc(in_dt_name=None):
    if in_dt_name is None:
        in_dt_name = IN_DT_NAME
    in_dt = getattr(mybir.dt, in_dt_name)

    nc = bacc.Bacc(trn_type="TRN2", target_bir_lowering=False, debug=False)

    # per-core inputs (host-packed; see make_in_maps)
    xq = nc.dram_tensor("xq", [P, DT, S], in_dt, kind="ExternalInput").ap()
    xk = nc.dram_tensor("xk", [P, DT, S], in_dt, kind="ExternalInput").ap()
    xv = nc.dram_tensor("xv", [P, DT, S], in_dt, kind="ExternalInput").ap()
    bbb = nc.dram_tensor("bbb", [P, JT, S], in_dt, kind="ExternalInput").ap()
    wq = nc.dram_tensor("wq", [P, JT, DT, P], in_dt, kind="ExternalInput").ap()
    wk = nc.dram_tensor("wk", [P, JT, DT, P], in_dt, kind="ExternalInput").ap()
    wv = nc.dram_tensor("wv", [P, JT, DT, P], in_dt, kind="ExternalInput").ap()
    wo = nc.dram_tensor("wo", [P, JT, H_DIM], in_dt, kind="ExternalInput").ap()
    # merged biases: rows = (bq, bk, nbq, nbk, bvl)
    b5 = nc.dram_tensor("b5", [P, 5, JT], F32, kind="ExternalInput").ap()
    out = nc.dram_tensor("out", [H_DIM, S], in_dt, kind="ExternalOutput").ap()

    from contextlib import ExitStack

    M = mybir.AluOpType

    with tile.TileContext(nc) as tcx, ExitStack() as ctx:
        wpool = ctx.enter_context(tcx.tile_pool(name="weights", bufs=1))
        xpool = ctx.enter_context(tcx.tile_pool(name="xin", bufs=1))
        ipool = ctx.enter_context(tcx.tile_pool(name="inter", bufs=2))
        ipool1 = ctx.enter_context(tcx.tile_pool(name="inter1", bufs=1))
        spool = ctx.enter_context(tcx.tile_pool(name="scan", bufs=1))
        cpool = ctx.enter_context(tcx.tile_pool(name="scol", bufs=2))
        ypool = ctx.enter_context(tcx.tile_pool(name="ysb", bufs=2))
        opool = ctx.enter_context(tcx.tile_pool(name="osb", bufs=4))
        ppool = ctx.enter_context(tcx.tile_pool(name="ps", bufs=4, space="PSUM"))

        # --- persistent weights / constants ---
        wk_sb = wpool.tile([P, JT, DT, P], in_dt, tag="wk")
        wv_sb = wpool.tile([P, JT, DT, P], in_dt, tag="wv")
        wq_sb = wpool.tile([P, JT, DT, P], in_dt, tag="wq")
        wo_sb = wpool.tile([P, JT, H_DIM], in_dt, tag="wo")
        b5_sb = wpool.tile([P, 5, JT], F32, tag="b5")

        def bap(i, lt):  # [P,1] bias slice
            return b5_sb[:, i, lt:lt + 1]

        # force the ACT exp-table DMA onto the queue BEFORE the big input
        # loads (otherwise it lands ~20us in and stalls every activation)
        warm = wpool.tile([1, 1], F32, tag="warm")
        nc.vector.memset(warm[:], 0.0)
        warm2 = wpool.tile([1, 1], F32, tag="warm2")
        nc.scalar.activation(out=warm2[:], in_=warm[:], func=AF.Exp)

        s_prev = [None] * JT    # per-lt last-column state of previous half

        def load_x(src, tag, h):
            """Per-512-chunk tiles for fine-grained DMA->matmul overlap."""
            tl = []
            for c in range(CPH):
                t_ = xpool.tile([P, DT, TC], in_dt, tag=f"{tag}{c}")
                nc.sync.dma_start(out=t_[:], in_=src[:, :, ts(h * CPH + c, TC)])
                tl.append(t_)
            return tl

        for h in range(NH):
            hsl = ts(h, SH)

            if h == 0:
                # --- head: finest-first so the first k group starts ASAP ---
                xk_c0 = xpool.tile([P, DT, TC], in_dt, tag="xk0", name="xk_c0")
                xk_c = [xk_c0]
                nc.sync.dma_start(out=xk_c0[:, 0:4], in_=xk[:, 0:4, 0:TC])
                nc.sync.dma_start(out=wk_sb[:, 0], in_=wk[:, 0])
                nc.sync.dma_start(out=xk_c0[:, 4:DT], in_=xk[:, 4:DT, 0:TC])
                nc.sync.dma_start(out=wk_sb[:, 1:JT], in_=wk[:, 1:JT])
                t_ = xpool.tile([P, DT, TC], in_dt, tag="xk1")
                nc.sync.dma_start(out=t_[:], in_=xk[:, :, TC:SH])
                xk_c.append(t_)
                nc.sync.dma_start(out=b5_sb[:], in_=b5)
                bb_c = xpool.tile([P, JT, SH], in_dt, tag="bbb")
                nc.sync.dma_start(out=bb_c[:], in_=bbb[:, :, hsl])
                nc.sync.dma_start(out=wv_sb[:], in_=wv)
                xv_c = load_x(xv, "xv", 0)
                nc.sync.dma_start(out=wq_sb[:], in_=wq)
                xq_c = load_x(xq, "xq", 0)
                nc.sync.dma_start(out=wo_sb[:], in_=wo)

            # ---- phase 1: k projections + gate chain per lane ----
            # DVE order [pk0,w0,pk1,w1,a0,pk2,w2,a1,pk3,w3,a2,a3] keeps the
            # gpsimd g-ops off DVE's critical path while finishing all gates
            # within the k phase.
            w_t = [None] * JT
            a_t = [None] * JT
            g_t = [None] * JT
            pend_a = []

            def emit_a(lt):
                a = ipool1.tile([P, SH], F32, tag=f"a{lt}", name=f"a{lt}")
                nc.vector.tensor_scalar(out=a[:], in0=g_t[lt][:],
                                        scalar1=-1.0, scalar2=1.0,
                                        op0=M.mult, op1=M.add)
                a_t[lt] = a

            for lt in range(JT):
                psk = ppool.tile([P, CPH, TC], F32, tag="ps")
                for c in range(CPH):
                    for d in range(DT):
                        nc.tensor.matmul(
                            out=psk[:, c], lhsT=wk_sb[:, lt, d],
                            rhs=xk_c[c][:, d],
                            start=(d == 0), stop=(d == DT - 1),
                        )
                rk = ipool1.tile([P, SH], F32, tag="rpos")
                nc.scalar.activation(out=rk[:], in_=psk[:], func=AF.Relu,
                                     bias=bap(1, lt))
                r2k = ipool1.tile([P, SH], F32, tag="rneg")
                nc.scalar.activation(out=r2k[:], in_=psk[:], func=AF.Relu,
                                     bias=bap(3, lt), scale=-1.0)
                ek = ipool1.tile([P, SH], F32, tag="ex")
                nc.scalar.activation(out=ek[:], in_=r2k[:], func=AF.Exp,
                                     scale=-1.0)
                pk = ipool.tile([P, SH], F32, tag="pk")
                nc.vector.tensor_tensor(out=pk[:], in0=ek[:], in1=rk[:],
                                        op=M.add)
                w = ipool1.tile([P, SH], F32, tag=f"w{lt}")
                nc.vector.tensor_tensor(out=w[:], in0=pk[:], in1=bb_c[:, lt],
                                        op=M.mult)
                g = ipool.tile([P, SH], F32, tag="g")
                nc.gpsimd.tensor_tensor(out=g[:], in0=pk[:], in1=w[:],
                                        op=M.mult)
                w_t[lt] = w
                g_t[lt] = g
                if lt >= 1:
                    emit_a(lt - 1)
            emit_a(JT - 1)

            # all xk / bbb reads emitted; prefetch next half's behind them
            if h < NH - 1:
                xk_n = load_x(xk, "xk", h + 1)
                bb_n = xpool.tile([P, JT, SH], in_dt, tag="bbb")
                nc.sync.dma_start(out=bb_n[:], in_=bbb[:, :, ts(h + 1, SH)])

            # ---- phase 2a: v projections + scans for all lanes ----
            s_t = [None] * JT
            for lt in range(JT):
                psv = ppool.tile([P, CPH, TC], F32, tag="ps")
                for d in range(DT):
                    for c in range(CPH):
                        nc.tensor.matmul(
                            out=psv[:, c], lhsT=wv_sb[:, lt, d],
                            rhs=xv_c[c][:, d],
                            start=(d == 0), stop=(d == DT - 1),
                        )
                cc = ipool.tile([P, SH], F32, tag="cc")
                nc.vector.scalar_tensor_tensor(
                    out=cc[:], in0=psv[:], scalar=bap(4, lt),
                    in1=w_t[lt][:], op0=M.add, op1=M.mult)
                s_new = spool.tile([P, SH], F32, tag=f"s{lt}")
                init = 0.0 if h == 0 else s_prev[lt][:]
                nc.vector.tensor_tensor_scan(
                    out=s_new[:], data0=a_t[lt][:], data1=cc[:], initial=init,
                    op0=M.mult, op1=M.add,
                )
                s_t[lt] = s_new
                if h < NH - 1:
                    scol = cpool.tile([P, 1], F32, tag=f"sc{lt}")
                    nc.vector.tensor_scalar(out=scol[:],
                                            in0=s_new[:, SH - 1:SH],
                                            scalar1=0.0, scalar2=None,
                                            op0=M.add)
                    s_prev[lt] = scol

            if h < NH - 1:
                xv_n = load_x(xv, "xv", h + 1)

            # ---- phase 2b: q projections + phi(q) + y per lane ----
            y_t = []
            for lt in range(JT):
                psq = ppool.tile([P, CPH, TC], F32, tag="ps")
                for d in range(DT):
                    for c in range(CPH):
                        nc.tensor.matmul(
                            out=psq[:, c], lhsT=wq_sb[:, lt, d],
                            rhs=xq_c[c][:, d],
                            start=(d == 0), stop=(d == DT - 1),
                        )
                rq = ipool.tile([P, SH], BF16, tag="rposq")
                nc.scalar.activation(out=rq[:], in_=psq[:], func=AF.Relu,
                                     bias=bap(0, lt))
                r2q = ipool1.tile([P, SH], BF16, tag="rnegq")
                nc.scalar.activation(out=r2q[:], in_=psq[:], func=AF.Relu,
                                     bias=bap(2, lt), scale=-1.0)
                eq = ipool1.tile([P, SH], BF16, tag="exq")
                nc.scalar.activation(out=eq[:], in_=r2q[:], func=AF.Exp,
                                     scale=-1.0)
                pq = ipool1.tile([P, SH], BF16, tag="pq")
                nc.vector.tensor_tensor(out=pq[:], in0=eq[:], in1=rq[:],
                                        op=M.add)
                y = ypool.tile([P, CPH, TC], in_dt, tag=f"y{lt}")
                nc.vector.tensor_tensor(out=y[:], in0=s_t[lt][:], in1=pq[:],
                                        op=M.mult)
                y_t.append(y)

            if h < NH - 1:
                xq_n = load_x(xq, "xq", h + 1)

            # ---- O projection in ot-pairs, lt=3 deferred ----
            for op_ in range(DT // 2):
                pts = []
                for ot in (2 * op_, 2 * op_ + 1):
                    pso = ppool.tile([P, CPH, TC], F32, tag="ps")
                    for lt in range(JT - 1):
                        for c in range(CPH):
                            nc.tensor.matmul(
                                out=pso[:, c], lhsT=wo_sb[:, lt, ts(ot, P)],
                                rhs=y_t[lt][:, c],
                                start=(lt == 0), stop=False,
                            )
                    pts.append(pso)
                for i, ot in enumerate((2 * op_, 2 * op_ + 1)):
                    pso = pts[i]
                    for c in range(CPH):
                        nc.tensor.matmul(
                            out=pso[:, c], lhsT=wo_sb[:, JT - 1, ts(ot, P)],
                            rhs=y_t[JT - 1][:, c],
                            start=False, stop=True,
                        )
                    o_sb = opool.tile([P, SH], in_dt, tag="osb")
                    if ot % 2 == 0:
                        nc.scalar.copy(out=o_sb[:], in_=pso[:])
                    else:
                        nc.vector.tensor_scalar(out=o_sb[:], in0=pso[:],
                                                scalar1=0.0, scalar2=None,
                                                op0=M.add)
                    nc.sync.dma_start(out=out[ts(ot, P), hsl], in_=o_sb[:])

            if h < NH - 1:
                xk_c, xv_c, bb_c, xq_c = xk_n, xv_n, bb_n, xq_n

    nc.compile()
    return nc


_NC_CACHE = {}


def _get_nc():
    key = IN_DT_NAME
    if key not in _NC_CACHE:
        _NC_CACHE[key] = build_nc()
    return _NC_CACHE[key]


def _np_in_dt():
    if IN_DT_NAME == "bfloat16":
        import ml_dtypes
        return ml_dtypes.bfloat16
    return np.float32


def make_in_maps(query, key, value, beta, Wq, bq, Wk, bk, Wv, bv, Wb, bb, Wo, bo):
    """Host-side shard prep: core_id = b*2 + hg."""
    ndt = _np_in_dt()

    def xpack(x):  # [S, H_DIM] -> [p, dt, t] in in_dt
        a = np.asarray(x, np.float32).T            # [H_DIM, S] = [dt*128+p, t]
        a = a.reshape(DT, P, S).transpose(1, 0, 2)  # [p, dt, t]
        return np.ascontiguousarray(a).astype(ndt)

    def wpack(Wsl):  # [J, H_DIM] -> [p, lt, dt, m]
        a = np.asarray(Wsl, np.float32)            # [lt*128+m, dt*128+p]
        a = a.reshape(JT, P, DT, P)                # [lt, m, dt, p]
        a = a.transpose(3, 0, 2, 1)                # [p, lt, dt, m]
        return np.ascontiguousarray(a).astype(ndt)

    xqs = [xpack(query[b]) for b in range(B)]
    xks = [xpack(key[b]) for b in range(B)]
    xvs = [xpack(value[b]) for b in range(B)]
    # gate b computed host-side (0.4% of FLOPs), pre-broadcast per lane
    Wbf = np.asarray(Wb, np.float32)
    bbf0 = np.asarray(bb, np.float32)
    z = np.einsum('bsd,hd->bsh', np.asarray(beta, np.float32), Wbf) + bbf0
    bgate = 1.0 / (1.0 + np.exp(-z))                      # [B, S, 16]

    def bpack(bl):  # [S, J] -> [p, lt, t]
        a = bl.T.reshape(JT, P, S)                        # [lt, p, t]
        return np.ascontiguousarray(a.transpose(1, 0, 2)).astype(ndt)

    bqf = np.asarray(bq, np.float32)
    bkf = np.asarray(bk, np.float32)
    bvf = np.asarray(bv, np.float32)

    in_maps = []
    for b in range(B):
        for hg in range(HG):
            jsl = slice(hg * J, (hg + 1) * J)
            hsl = slice(hg * HPC, (hg + 1) * HPC)

            def lanes(v):  # [J] -> [128, 4] per lane-tile columns
                return np.ascontiguousarray(v[jsl].reshape(JT, P).T)

            b5m = np.stack([lanes(bqf), lanes(bkf), lanes(-bqf),
                            lanes(-bkf), lanes(bvf)], axis=1)  # [P,5,JT]

            wop = np.asarray(Wo, np.float32)[:, jsl]      # [o, j]
            wop = wop.T.reshape(JT, P, H_DIM).transpose(1, 0, 2)  # [p, lt, o]

            in_maps.append({
                "xq": xqs[b], "xk": xks[b], "xv": xvs[b],
                "bbb": bpack(np.repeat(bgate[b][:, hsl], HEAD_DIM, axis=1)),
                "wq": wpack(Wq[jsl]), "wk": wpack(Wk[jsl]),
                "wv": wpack(Wv[jsl]),
                "wo": np.ascontiguousarray(wop).astype(ndt),
                "b5": np.ascontiguousarray(b5m),
            })
    return in_maps


LAST_RESULTS = None


def kernel(**inputs):
    global LAST_RESULTS
    nc = _get_nc()
    in_maps = make_in_maps(**inputs)
    res = run_bass_kernel_spmd(nc, in_maps, core_ids=list(range(NCORES)),
                               trace=bool(os.environ.get("DELTA_TRACE")))
    LAST_RESULTS = res
    bo = np.asarray(inputs["bo"], np.float32)
    out = np.empty((B, S, H_DIM), np.float32)
    for b in range(B):
        m = (np.asarray(res.results[2 * b]["out"], np.float32)
             + np.asarray(res.results[2 * b + 1]["out"], np.float32))
        out[b] = m.T + bo
    return out


# revision 29
# speedup vs baseline: 1.1614x; 1.0020x over previous
"""DeltaRule (diagonal-state linear attention) Bass kernel for 8 TRN2 cores.

Problem: nn_DeltaRule_20194936225992
  B=4, S=2048, H_DIM=1024, N_HEADS=16, HEAD_DIM=64.
  q/k/v/b projections, phi = elu+1, per-(b,h,d) scalar linear recurrence
      s_t = (1 - b_t*pk_t^2) * s_{t-1} + b_t*v_t*pk_t ;  y_t = s_t * pq_t
  out = y @ Wo.T + bo

Sharding: core = (batch b, head-group hg) with hg covering 8 heads.
Each core computes its partial O-projection over its 512 lanes; host
sums the two bf16 head-group partials per batch, transposes and adds bo.

v4 design: time processed in 2 halves of SH=1024, elementwise ops on
[128,1024] tiles.  PE schedule per half:
  1. k-projections for all 4 lane-tiles (PSUM groups [128,2,512],
     c-outer so the first group only needs the first 512-col x chunk).
     The FULL gate chain (phi(k), w=pk*b, g=pk*w on gpsimd, a=1-g)
     drains per lane-tile right here, so by the v/q phase only cc and
     the scan remain on the critical path.
  2. v/q pairs per lane-tile: cc=(psv+bv)*w fused from PSUM, scan,
     phi(q) in bf16, y=s*pq.
  3. O-projection in ot-pairs with the lt=3 contribution deferred
     (last y stays off the critical path).  Copies alternate ACT/DVE.
DMA: head order puts xk chunk 0 first, then wk (single dispatch), then
the rest; biases ride in one merged [P,5,JT] tensor; next half's x
prefetches are emitted right after the last reader of this half's.
PE steady state measured ~216 ns per 512-col matmul (ldweights fully
overlapped) == bf16 roofline.
"""

import os
import sys

for _p in ("/opt/trn_rl_repo", os.path.expanduser("~/.axon_site/_ro/trn_rl_repo")):
    if os.path.isdir(_p) and _p not in sys.path:
        sys.path.insert(0, _p)

import numpy as np  # noqa: E402

import concourse.bass as bass  # noqa: E402
import concourse.tile as tile  # noqa: E402
from concourse import bacc, mybir  # noqa: E402
from concourse.bass import ts  # noqa: E402
from concourse.bass_utils import run_bass_kernel_spmd  # noqa: E402

# problem constants (hardcoded per task rules)
B, S, H_DIM, N_HEADS, HEAD_DIM = 4, 2048, 1024, 16, 64
P = 128
NCORES = 8
HG = 2                      # head groups
J = 512                     # lanes per core  (8 heads * 64)
JT = J // P                 # 4 j-tiles
DT = H_DIM // P             # 8 contraction tiles
HPC = N_HEADS // HG         # 8 heads per core
SH = 1024                   # half length
NH = S // SH                # 2 halves
TC = 512                    # psum chunk (1 bank)
CPH = SH // TC              # chunks per half = 2

IN_DT_NAME = os.environ.get("DELTA_IN_DT", "bfloat16")

F32 = mybir.dt.float32
BF16 = mybir.dt.bfloat16
AF = mybir.ActivationFunctionType


def build_nc(in_dt_name=None):
    if in_dt_name is None:
        in_dt_name = IN_DT_NAME
    in_dt = getattr(mybir.dt, in_dt_name)

    nc = bacc.Bacc(trn_type="TRN2", target_bir_lowering=False, debug=False)

    # per-core inputs (host-packed; see make_in_maps)
    xq = nc.dram_tensor("xq", [P, DT, S], in_dt, kind="ExternalInput").ap()
    xk = nc.dram_tensor("xk", [P, DT, S], in_dt, kind="ExternalInput").ap()
    xv = nc.dram_tensor("xv", [P, DT, S], in_dt, kind="ExternalInput").ap()
    bbb = nc.dram_tensor("bbb", [P, JT, S], in_dt, kind="ExternalInput").ap()
    wq = nc.dram_tensor("wq", [P, JT, DT, P], in_dt, kind="ExternalInput").ap()
    wk = nc.dram_tensor("wk", [P, JT, DT, P], in_dt, kind="ExternalInput").ap()
    wv = nc.dram_tensor("wv", [P, JT, DT, P], in_dt, kind="ExternalInput").ap()
    wo = nc.dram_tensor("wo", [P, JT, H_DIM], in_dt, kind="ExternalInput").ap()
    # merged biases: rows = (bq, bk, nbq, nbk, bvl)
    b5 = nc.dram_tensor("b5", [P, 5, JT], F32, kind="ExternalInput").ap()
    out = nc.dram_tensor("out", [H_DIM, S], in_dt, kind="ExternalOutput").ap()

    from contextlib import ExitStack

    M = mybir.AluOpType

    with tile.TileContext(nc) as tcx, ExitStack() as ctx:
        wpool = ctx.enter_context(tcx.tile_pool(name="weights", bufs=1))
        xpool = ctx.enter_context(tcx.tile_pool(name="xin", bufs=1))
        ipool = ctx.enter_context(tcx.tile_pool(name="inter", bufs=2))
        ipool1 = ctx.enter_context(tcx.tile_pool(name="inter1", bufs=1))
        spool = ctx.enter_context(tcx.tile_pool(name="scan", bufs=1))
        cpool = ctx.enter_context(tcx.tile_pool(name="scol", bufs=2))
        ypool = ctx.enter_context(tcx.tile_pool(name="ysb", bufs=2))
        opool = ctx.enter_context(tcx.tile_pool(name="osb", bufs=4))
        ppool = ctx.enter_context(tcx.tile_pool(name="ps", bufs=4, space="PSUM"))

        # --- persistent weights / constants ---
        wk_sb = wpool.tile([P, JT, DT, P], in_dt, tag="wk")
        wv_sb = wpool.tile([P, JT, DT, P], in_dt, tag="wv")
        wq_sb = wpool.tile([P, JT, DT, P], in_dt, tag="wq")
        wo_sb = wpool.tile([P, JT, H_DIM], in_dt, tag="wo")
        b5_sb = wpool.tile([P, 5, JT], F32, tag="b5")

        def bap(i, lt):  # [P,1] bias slice
            return b5_sb[:, i, lt:lt + 1]

        # force the ACT exp-table DMA onto the queue BEFORE the big input
        # loads (otherwise it lands ~20us in and stalls every activation)
        warm = wpool.tile([1, 1], F32, tag="warm")
        nc.vector.memset(warm[:], 0.0)
        warm2 = wpool.tile([1, 1], F32, tag="warm2")
        nc.scalar.activation(out=warm2[:], in_=warm[:], func=AF.Exp)

        s_prev = [None] * JT    # per-lt last-column state of previous half

        def load_x(src, tag, h):
            """Per-512-chunk tiles for fine-grained DMA->matmul overlap."""
            tl = []
            for c in range(CPH):
                t_ = xpool.tile([P, DT, TC], in_dt, tag=f"{tag}{c}")
                nc.sync.dma_start(out=t_[:], in_=src[:, :, ts(h * CPH + c, TC)])
                tl.append(t_)
            return tl

        for h in range(NH):
            hsl = ts(h, SH)

            if h == 0:
                # --- head: finest-first so the first k group starts ASAP ---
                nc.sync.dma_start(out=b5_sb[:], in_=b5)
                xk_c0 = xpool.tile([P, DT, TC], in_dt, tag="xk0", name="xk_c0")
                xk_c = [xk_c0]
                nc.sync.dma_start(out=xk_c0[:, 0:4], in_=xk[:, 0:4, 0:TC])
                nc.sync.dma_start(out=wk_sb[:, 0], in_=wk[:, 0])
                nc.sync.dma_start(out=xk_c0[:, 4:DT], in_=xk[:, 4:DT, 0:TC])
                nc.sync.dma_start(out=wk_sb[:, 1:JT], in_=wk[:, 1:JT])
                t_ = xpool.tile([P, DT, TC], in_dt, tag="xk1")
                nc.sync.dma_start(out=t_[:], in_=xk[:, :, TC:SH])
                xk_c.append(t_)
                bb_c = xpool.tile([P, JT, SH], in_dt, tag="bbb")
                nc.sync.dma_start(out=bb_c[:], in_=bbb[:, :, hsl])
                nc.sync.dma_start(out=wv_sb[:], in_=wv)
                xv_c = load_x(xv, "xv", 0)
                nc.sync.dma_start(out=wq_sb[:], in_=wq)
                xq_c = load_x(xq, "xq", 0)
                nc.sync.dma_start(out=wo_sb[:], in_=wo)

            # ---- phase 1: k projections + gate chain per lane (all DVE;
            # cross-engine hops through gpsimd proved 5-10us slower) ----
            w_t = [None] * JT
            a_t = [None] * JT

            for lt in range(JT):
                psk = ppool.tile([P, CPH, TC], F32, tag="ps")
                for c in range(CPH):
                    for d in range(DT):
                        nc.tensor.matmul(
                            out=psk[:, c], lhsT=wk_sb[:, lt, d],
                            rhs=xk_c[c][:, d],
                            start=(d == 0), stop=(d == DT - 1),
                        )
                rk = ipool1.tile([P, SH], F32, tag="rpos")
                nc.scalar.activation(out=rk[:], in_=psk[:], func=AF.Relu,
                                     bias=bap(1, lt))
                r2k = ipool1.tile([P, SH], F32, tag="rneg")
                nc.scalar.activation(out=r2k[:], in_=psk[:], func=AF.Relu,
                                     bias=bap(3, lt), scale=-1.0)
                ek = ipool1.tile([P, SH], F32, tag="ex")
                nc.scalar.activation(out=ek[:], in_=r2k[:], func=AF.Exp,
                                     scale=-1.0)
                pk = ipool.tile([P, SH], F32, tag="pk")
                nc.vector.tensor_tensor(out=pk[:], in0=ek[:], in1=rk[:],
                                        op=M.add)
                w = ipool1.tile([P, SH], F32, tag=f"w{lt}")
                nc.vector.tensor_tensor(out=w[:], in0=pk[:], in1=bb_c[:, lt],
                                        op=M.mult)
                g = ipool.tile([P, SH], F32, tag="g")
                nc.vector.tensor_tensor(out=g[:], in0=pk[:], in1=w[:],
                                        op=M.mult)
                a = ipool1.tile([P, SH], F32, tag=f"a{lt}", name=f"a{lt}")
                nc.vector.tensor_scalar(out=a[:], in0=g[:],
                                        scalar1=-1.0, scalar2=1.0,
                                        op0=M.mult, op1=M.add)
                w_t[lt] = w
                a_t[lt] = a

            # all xk / bbb reads emitted; prefetch next half's behind them
            if h < NH - 1:
                xk_n = load_x(xk, "xk", h + 1)
                bb_n = xpool.tile([P, JT, SH], in_dt, tag="bbb")
                nc.sync.dma_start(out=bb_n[:], in_=bbb[:, :, ts(h + 1, SH)])

            # ---- phase 2a: v projections + scans for all lanes ----
            s_t = [None] * JT
            for lt in range(JT):
                psv = ppool.tile([P, CPH, TC], F32, tag="ps")
                for d in range(DT):
                    for c in range(CPH):
                        nc.tensor.matmul(
                            out=psv[:, c], lhsT=wv_sb[:, lt, d],
                            rhs=xv_c[c][:, d],
                            start=(d == 0), stop=(d == DT - 1),
                        )
                cc = ipool.tile([P, SH], F32, tag="cc")
                nc.vector.scalar_tensor_tensor(
                    out=cc[:], in0=psv[:], scalar=bap(4, lt),
                    in1=w_t[lt][:], op0=M.add, op1=M.mult)
                s_new = spool.tile([P, SH], F32, tag=f"s{lt}")
                init = 0.0 if h == 0 else s_prev[lt][:]
                nc.vector.tensor_tensor_scan(
                    out=s_new[:], data0=a_t[lt][:], data1=cc[:], initial=init,
                    op0=M.mult, op1=M.add,
                )
                s_t[lt] = s_new
                if h < NH - 1:
                    scol = cpool.tile([P, 1], F32, tag=f"sc{lt}")
                    nc.gpsimd.tensor_scalar(out=scol[:],
                                            in0=s_new[:, SH - 1:SH],
                                            scalar1=0.0, scalar2=None,
                                            op0=M.add)
                    s_prev[lt] = scol

            if h < NH - 1:
                xv_n = load_x(xv, "xv", h + 1)

            # ---- phase 2b: q projections + phi(q) + y per lane ----
            y_t = []
            for lt in range(JT):
                psq = ppool.tile([P, CPH, TC], F32, tag="ps")
                for d in range(DT):
                    for c in range(CPH):
                        nc.tensor.matmul(
                            out=psq[:, c], lhsT=wq_sb[:, lt, d],
                            rhs=xq_c[c][:, d],
                            start=(d == 0), stop=(d == DT - 1),
                        )
                rq = ipool.tile([P, SH], BF16, tag="rposq")
                nc.scalar.activation(out=rq[:], in_=psq[:], func=AF.Relu,
                                     bias=bap(0, lt))
                r2q = ipool1.tile([P, SH], BF16, tag="rnegq")
                nc.scalar.activation(out=r2q[:], in_=psq[:], func=AF.Relu,
                                     bias=bap(2, lt), scale=-1.0)
                eq = ipool1.tile([P, SH], BF16, tag="exq")
                nc.scalar.activation(out=eq[:], in_=r2q[:], func=AF.Exp,
                                     scale=-1.0)
                pq = ipool1.tile([P, SH], BF16, tag="pq")
                eng = nc.vector if lt == JT - 1 else nc.gpsimd
                eng.tensor_tensor(out=pq[:], in0=eq[:], in1=rq[:], op=M.add)
                y = ypool.tile([P, CPH, TC], in_dt, tag=f"y{lt}")
                nc.vector.tensor_tensor(out=y[:], in0=s_t[lt][:], in1=pq[:],
                                        op=M.mult)
                y_t.append(y)

            if h < NH - 1:
                xq_n = load_x(xq, "xq", h + 1)

            # ---- O projection in ot-pairs, lt=3 deferred ----
            for op_ in range(DT // 2):
                pts = []
                for ot in (2 * op_, 2 * op_ + 1):
                    pso = ppool.tile([P, CPH, TC], F32, tag="ps")
                    for lt in range(JT - 1):
                        for c in range(CPH):
                            nc.tensor.matmul(
                                out=pso[:, c], lhsT=wo_sb[:, lt, ts(ot, P)],
                                rhs=y_t[lt][:, c],
                                start=(lt == 0), stop=False,
                            )
                    pts.append(pso)
                for i, ot in enumerate((2 * op_, 2 * op_ + 1)):
                    pso = pts[i]
                    for c in range(CPH):
                        nc.tensor.matmul(
                            out=pso[:, c], lhsT=wo_sb[:, JT - 1, ts(ot, P)],
                            rhs=y_t[JT - 1][:, c],
                            start=False, stop=True,
                        )
                    o_sb = opool.tile([P, SH], in_dt, tag="osb")
                    if ot % 2 == 0:
                        nc.scalar.copy(out=o_sb[:], in_=pso[:])
                    else:
                        nc.vector.tensor_scalar(out=o_sb[:], in0=pso[:],
                                                scalar1=0.0, scalar2=None,
                                                op0=M.add)
                    nc.sync.dma_start(out=out[ts(ot, P), hsl], in_=o_sb[:])

            if h < NH - 1:
                xk_c, xv_c, bb_c, xq_c = xk_n, xv_n, bb_n, xq_n

    nc.compile()
    return nc


_NC_CACHE = {}


def _get_nc():
    key = IN_DT_NAME
    if key not in _NC_CACHE:
        _NC_CACHE[key] = build_nc()
    return _NC_CACHE[key]


def _np_in_dt():
    if IN_DT_NAME == "bfloat16":
        import ml_dtypes
        return ml_dtypes.bfloat16
    return np.float32


def make_in_maps(query, key, value, beta, Wq, bq, Wk, bk, Wv, bv, Wb, bb, Wo, bo):
    """Host-side shard prep: core_id = b*2 + hg."""
    ndt = _np_in_dt()

    def xpack(x):  # [S, H_DIM] -> [p, dt, t] in in_dt
        a = np.asarray(x, np.float32).T            # [H_DIM, S] = [dt*128+p, t]
        a = a.reshape(DT, P, S).transpose(1, 0, 2)  # [p, dt, t]
        return np.ascontiguousarray(a).astype(ndt)

    def wpack(Wsl):  # [J, H_DIM] -> [p, lt, dt, m]
        a = np.asarray(Wsl, np.float32)            # [lt*128+m, dt*128+p]
        a = a.reshape(JT, P, DT, P)                # [lt, m, dt, p]
        a = a.transpose(3, 0, 2, 1)                # [p, lt, dt, m]
        return np.ascontiguousarray(a).astype(ndt)

    xqs = [xpack(query[b]) for b in range(B)]
    xks = [xpack(key[b]) for b in range(B)]
    xvs = [xpack(value[b]) for b in range(B)]
    # gate b computed host-side (0.4% of FLOPs), pre-broadcast per lane
    Wbf = np.asarray(Wb, np.float32)
    bbf0 = np.asarray(bb, np.float32)
    z = np.einsum('bsd,hd->bsh', np.asarray(beta, np.float32), Wbf) + bbf0
    bgate = 1.0 / (1.0 + np.exp(-z))                      # [B, S, 16]

    def bpack(bl):  # [S, J] -> [p, lt, t]
        a = bl.T.reshape(JT, P, S)                        # [lt, p, t]
        return np.ascontiguousarray(a.transpose(1, 0, 2)).astype(ndt)

    bqf = np.asarray(bq, np.float32)
    bkf = np.asarray(bk, np.float32)
    bvf = np.asarray(bv, np.float32)

    in_maps = []
    for b in range(B):
        for hg in range(HG):
            jsl = slice(hg * J, (hg + 1) * J)
            hsl = slice(hg * HPC, (hg + 1) * HPC)

            def lanes(v):  # [J] -> [128, 4] per lane-tile columns
                return np.ascontiguousarray(v[jsl].reshape(JT, P).T)

            b5m = np.stack([lanes(bqf), lanes(bkf), lanes(-bqf),
                            lanes(-bkf), lanes(bvf)], axis=1)  # [P,5,JT]

            wop = np.asarray(Wo, np.float32)[:, jsl]      # [o, j]
            wop = wop.T.reshape(JT, P, H_DIM).transpose(1, 0, 2)  # [p, lt, o]

            in_maps.append({
                "xq": xqs[b], "xk": xks[b], "xv": xvs[b],
                "bbb": bpack(np.repeat(bgate[b][:, hsl], HEAD_DIM, axis=1)),
                "wq": wpack(Wq[jsl]), "wk": wpack(Wk[jsl]),
                "wv": wpack(Wv[jsl]),
                "wo": np.ascontiguousarray(wop).astype(ndt),
                "b5": np.ascontiguousarray(b5m),
            })
    return in_maps


LAST_RESULTS = None


def kernel(**inputs):
    global LAST_RESULTS
    nc = _get_nc()
    in_maps = make_in_maps(**inputs)
    res = run_bass_kernel_spmd(nc, in_maps, core_ids=list(range(NCORES)),
                               trace=bool(os.environ.get("DELTA_TRACE")))
    LAST_RESULTS = res
    bo = np.asarray(inputs["bo"], np.float32)
    out = np.empty((B, S, H_DIM), np.float32)
    for b in range(B):
        m = (np.asarray(res.results[2 * b]["out"], np.float32)
             + np.asarray(res.results[2 * b + 1]["out"], np.float32))
        out[b] = m.T + bo
    return out
